# revision 11
# baseline (speedup 1.0000x reference)
"""Trainium2 Bass kernel for nn_Att_R (low-rank attention + scatter Laplacian).

Data-parallel over 8 NeuronCores: each core handles 1024 batch rows = 16 envs.
b-assignment within a core: b = 64*e + 16*g + 4*tau + s  (g,s in 0..3, tau in 0..3)
so that U's partition index 32g + 8tau + 2s + pr falls out of a 32x32
StreamTranspose of the Rh^2 tile.
"""

import numpy as np


def _patch_tile_drain():
    """walrus in this container rejects multi-wait DRAIN instructions
    (Too many sync wait commands); split the tail-drain waits across
    single-wait NOPs instead."""
    import concourse.mybir as mybir
    from concourse import tile
    from concourse.vector_clock import ScopedClock

    if getattr(tile.TileContext, "_drain_patched", False):
        return

    def patched(self, tick_clock, wait_clock):
        probe = self.nc.sync.nop()
        wait_clock.add_sem_waits(probe.ins, ScopedClock({None: tick_clock.global_clock}))
        si = probe.ins.sync_info
        waits = list(si.on_wait)
        probe.ins.sync_info = mybir.SyncInfo(
            on_wait=waits[:1], on_update=list(si.on_update)
        )
        for i in range(1, len(waits)):
            n2 = self.nc.sync.nop()
            n2.ins.sync_info = mybir.SyncInfo(on_wait=[waits[i]], on_update=[])
        self.nc.sync.drain()
        self.nc.all_engine_barrier()
        popped = self.nc._tile_sem_poison_stack.pop()
        assert popped is self._sem_poison
        self.nc.clear_and_free_semaphores(list(self.sems.allocated().values()))
        self.nc.all_engine_barrier()

    tile.TileContext._drain_and_barrier = patched
    tile.TileContext._drain_patched = True


def _split_excess_waits(nc, cap=1):
    """walrus here rejects instructions carrying more than one sync-wait;
    hoist excess waits onto same-engine NOPs inserted just before."""
    import concourse.mybir as mybir

    cnt = 0
    for bbname, bassbb in nc.bb_map.items():
        bb = bassbb.bb
        new = []
        for inst in bb.instructions:
            si = getattr(inst, "sync_info", None)
            if si is None or len(si.on_wait) <= cap:
                new.append(inst)
                continue
            waits = list(si.on_wait)
            for w in waits[:-cap]:
                nop = mybir.InstNoOp(name=f"wsplit_{cnt}", ins=[], outs=[])
                cnt += 1
                nop.engine = inst.engine
                nop.sync_info = mybir.SyncInfo(on_wait=[w], on_update=[])
                nc.register_instruction(nop, overwrite=True)
                new.append(nop)
            inst.sync_info = mybir.SyncInfo(
                on_wait=waits[-cap:], on_update=list(si.on_update)
            )
            new.append(inst)
        bb.instructions = new


def build_graph():
    _patch_tile_drain()
    import concourse.bass as bass
    import concourse.mybir as mybir
    from concourse import tile
    from contextlib import ExitStack

    f32 = mybir.dt.float32
    bf16 = mybir.dt.bfloat16
    i32 = mybir.dt.int32
    AF = mybir.ActivationFunctionType
    OP = mybir.AluOpType

    BL, D, NA, R, H = 1024, 64, 64, 32, 16
    EL = BL // NA  # 16 envs per core

    nc = bass.Bass()
    x_ext = nc.declare_dram_parameter("x", (BL, D, NA), f32, isOutput=False)
    L_ext = nc.declare_dram_parameter("L", (BL, 1, NA), f32, isOutput=False)
    Aq_ext = nc.declare_dram_parameter("Aq", (R, D), f32, isOutput=False)
    Ak_ext = nc.declare_dram_parameter("Ak", (R, D), f32, isOutput=False)
    Av_ext = nc.declare_dram_parameter("Av", (R, D), f32, isOutput=False)
    Ao_ext = nc.declare_dram_parameter("Ao", (H, R), f32, isOutput=False)
    out_ext = nc.declare_dram_parameter("out", (EL, 256, 256), f32, isOutput=True)
    lam_dram = nc.dram_tensor("lam_scratch", (32, BL), f32)

    with tile.TileContext(nc) as tc, ExitStack() as ctx:
        cpool = ctx.enter_context(tc.tile_pool(name="consts", bufs=1))
        xpool = ctx.enter_context(tc.tile_pool(name="x", bufs=3))
        qkvpool = ctx.enter_context(tc.tile_pool(name="qkv", bufs=2))
        tpool = ctx.enter_context(tc.tile_pool(name="tq", bufs=2))
        spool = ctx.enter_context(tc.tile_pool(name="soft", bufs=2))
        opool = ctx.enter_context(tc.tile_pool(name="o", bufs=2))
        rhpool = ctx.enter_context(tc.tile_pool(name="rh", bufs=2))
        upool = ctx.enter_context(tc.tile_pool(name="u", bufs=2))
        espool = ctx.enter_context(tc.tile_pool(name="es", bufs=2))
        pp = ctx.enter_context(tc.tile_pool(name="ps", bufs=2, space="PSUM"))
        ppe = ctx.enter_context(tc.tile_pool(name="pse", bufs=1, space="PSUM"))

        # ---------- constants ----------
        Wprod = cpool.tile([128, 96], f32)  # [d, (q|k|v) r], replicated at 0-63 / 64-127
        for pb in (0, 64):
            nc.sync.dma_start(Wprod[pb : pb + 64, 0:32], Aq_ext[:].rearrange("r d -> d r"))
            nc.sync.dma_start(Wprod[pb : pb + 64, 32:64], Ak_ext[:].rearrange("r d -> d r"))
            nc.sync.dma_start(Wprod[pb : pb + 64, 64:96], Av_ext[:].rearrange("r d -> d r"))
        AoT = cpool.tile([128, 16], f32)
        for g in range(4):
            nc.sync.dma_start(
                AoT[32 * g : 32 * g + 32, :], Ao_ext[:].rearrange("h r -> r h")
            )

        iota_f = cpool.tile([128, 128], f32)
        nc.gpsimd.iota(iota_f[:], pattern=[[1, 128]], base=0, channel_multiplier=0,
                       allow_small_or_imprecise_dtypes=True)
        iota_p = cpool.tile([128, 1], f32)
        nc.gpsimd.iota(iota_p[:], pattern=[[0, 1]], base=0, channel_multiplier=1,
                       allow_small_or_imprecise_dtypes=True)
        ident = cpool.tile([128, 128], f32)
        nc.vector.tensor_scalar(
            out=ident[:], in0=iota_f[:], scalar1=iota_p[:, 0:1], scalar2=None,
            op0=OP.is_equal,
        )
        ident_bf = cpool.tile([128, 128], bf16)
        nc.vector.tensor_copy(ident_bf[:], ident[:])
        AoTb = cpool.tile([128, 16], bf16)
        nc.vector.tensor_copy(AoTb[:], AoT[:])

        # ---------- phase 0: lam = 1/sqrt(numN) ----------
        Lt = cpool.tile([128, 512], f32)
        nc.sync.dma_start(
            Lt[:].rearrange("p (c o n) -> p c o n", c=8, o=1),
            L_ext[:].rearrange("(c p) o n -> p c o n", c=8, p=128),
        )
        ge = cpool.tile([128, 512], f32)
        nc.vector.tensor_scalar(
            out=ge[:], in0=Lt[:], scalar1=1.0, scalar2=None, op0=OP.is_ge
        )
        numN = cpool.tile([128, 8], f32)
        nc.vector.tensor_reduce(
            out=numN[:], in_=ge[:].rearrange("p (c n) -> p c n", c=8),
            axis=mybir.AxisListType.X, op=OP.add,
        )
        nc.vector.tensor_scalar(
            out=numN[:], in0=numN[:], scalar1=1.0, scalar2=None, op0=OP.add
        )
        rinum = cpool.tile([128, 8], f32)
        nc.vector.reciprocal(rinum[:], numN[:])
        lam = cpool.tile([128, 8], f32)
        nc.scalar.activation(lam[:], rinum[:], AF.Sqrt)
        # replicate to DRAM: lam_dram[rep, b], b = 128c + p
        for rep in range(32):
            nc.sync.dma_start(
                lam_dram[rep].rearrange("(c p) -> p c", c=8, p=128), lam[:]
            )
        # reload: lam_sb[32g + i, (q:16, tau:4, s:4)], value = lam(64q+16g+4tau+s)
        lam_sb = cpool.tile([128, 256], f32)
        lam_v = lam_dram[:].rearrange(
            "rep (q g t s) -> g rep q t s", q=16, g=4, t=4, s=4
        )
        for g in range(4):
            nc.sync.dma_start(
                lam_sb[32 * g : 32 * g + 32, :].rearrange(
                    "p (q t s) -> p q t s", q=16, t=4
                ),
                lam_v[g],
            )

        x_v = x_ext[:].rearrange("(e g t s) d n -> e t d g s n", e=16, g=4, t=4, s=4)

        # ---------- main loop ----------
        for e in range(EL):
            o_sb = opool.tile([128, 1024], bf16, tag="o_sb")
            for tau in range(4):
                it = 4 * e + tau
                xt = xpool.tile([128, 1024], f32, tag="xt")
                for g in range(4):
                    pb, cb0 = (0, 256 * g) if g < 2 else (64, 256 * g - 512)
                    nc.sync.dma_start(
                        xt[pb : pb + 64, cb0 : cb0 + 256].rearrange(
                            "p (s n) -> p s n", s=4
                        ),
                        x_v[e, tau, :, g],
                    )

                qkv = qkvpool.tile([96, 1024], bf16, tag="qkv")
                for h in range(2):
                    ps = pp.tile([128, 512], f32, tag="ps_qkv")
                    nc.tensor.matmul(
                        out=ps[0:96, :],
                        lhsT=Wprod[64 * h : 64 * h + 64, :],
                        rhs=xt[64 * h : 64 * h + 64, 512 * h : 512 * h + 512],
                        start=True, stop=True,
                        tile_position=(64 * h, 0),
                    )
                    nc.scalar.activation(
                        qkv[:, 512 * h : 512 * h + 512], ps[0:96, :], AF.Tanh
                    )

                tq = tpool.tile([64, 1024], bf16, tag="tq")
                nc.vector.transpose(tq[:], qkv[0:64, :])
                ktq = tpool.tile([32, 1024], bf16, tag="ktq")
                nc.scalar.dma_start(ktq[:], tq[32:64, :])

                psE = pp.tile([128, 128], f32, tag="ps_E")
                for g in range(4):
                    for s in range(4):
                        cb = (4 * g + s) * 64
                        for hh in range(2):
                            nc.tensor.matmul(
                                out=psE[32 * g : 32 * g + 32, 32 * s : 32 * s + 32],
                                lhsT=tq[0:32, cb + 32 * hh : cb + 32 * hh + 32],
                                rhs=ktq[0:32, cb + 32 * hh : cb + 32 * hh + 32],
                                start=(hh == 0), stop=(hh == 1),
                                tile_position=(0, 32 * g),
                            )
                Esb = spool.tile([128, 128], bf16, tag="Esb")
                for s in range(4):
                    nc.scalar.activation(
                        Esb[:, 32 * s : 32 * s + 32], psE[:, 32 * s : 32 * s + 32],
                        AF.Exp, scale=lam_sb[:, 4 * it + s : 4 * it + s + 1],
                    )
                rs = spool.tile([128, 4], f32, tag="rs")
                nc.vector.tensor_reduce(
                    out=rs[:], in_=Esb[:].rearrange("p (s x) -> p s x", s=4),
                    axis=mybir.AxisListType.X, op=OP.add,
                )
                rsi = spool.tile([128, 4], f32, tag="rsi")
                nc.vector.reciprocal(rsi[:], rs[:])
                attn = spool.tile([128, 128], bf16, tag="attn")
                for s in range(4):
                    nc.vector.tensor_scalar(
                        out=attn[:, 32 * s : 32 * s + 32],
                        in0=Esb[:, 32 * s : 32 * s + 32],
                        scalar1=rsi[:, s : s + 1], scalar2=None, op0=OP.mult,
                    )
                attnT = spool.tile([128, 128], bf16, tag="attnT")
                nc.vector.transpose(attnT[:], attn[:])
                attnT2 = spool.tile([128, 512], bf16, tag="attnT2")
                for g in range(4):
                    nc.scalar.dma_start(
                        attnT2[64:96, 128 * g : 128 * g + 128],
                        attnT[32 * g : 32 * g + 32, :],
                    )

                psO = pp.tile([128, 256], f32, tag="ps_o")
                for g in range(4):
                    for s in range(4):
                        cb = (4 * g + s) * 64
                        nc.tensor.matmul(
                            out=psO[32 * g : 32 * g + 32, 64 * s : 64 * s + 64],
                            lhsT=attnT2[64:96, 128 * g + 32 * s : 128 * g + 32 * s + 32],
                            rhs=qkv[64:96, cb : cb + 64],
                            start=True, stop=True,
                            tile_position=(64, 32 * g),
                        )
                nc.scalar.activation(
                    o_sb[:, 256 * tau : 256 * tau + 256], psO[:], AF.Copy
                )

            # ---------- per-env tail ----------
            rh = rhpool.tile([128, 1024], bf16, tag="rh")
            for half in range(2):
                psR = ppe.tile([128, 512], f32, tag="ps_rh")
                for g in range(4):
                    nc.tensor.matmul(
                        out=psR[32 * g : 32 * g + 16, :],
                        lhsT=AoTb[32 * g : 32 * g + 32, :],
                        rhs=o_sb[32 * g : 32 * g + 32, 512 * half : 512 * half + 512],
                        start=True, stop=True,
                        tile_position=(32 * g, 32 * g),
                    )
                nc.scalar.activation(
                    rh[:, 512 * half : 512 * half + 512], psR[:], AF.Tanh
                )
            rh2 = rhpool.tile([128, 1024], bf16, tag="rh2")
            nc.vector.tensor_tensor(out=rh2[:], in0=rh[:], in1=rh[:], op=OP.mult)

            # U-build: rh2 free = (tau, s, nn), nn = 16j + 8hr + 4hc + 2pr + pc
            # transpose 32x32 blocks: in cols (pc-outer)(tau, s, pr) -> out (pc)(h)
            U = upool.tile([128, 1024], bf16, tag="U")
            rh2v = rh2[:].rearrange(
                "p (t s j h c r q) -> p h c j q t s r", t=4, s=4, j=4, h=2, c=2, r=2, q=2
            )
            Uv = U[:].rearrange(
                "p (Hh Cc ha hb hc2 j q) -> p Hh Cc j q ha hb hc2",
                Hh=2, Cc=2, ha=4, hb=4, hc2=2, j=4, q=2,
            )
            for hr in range(2):
                for hcc in range(2):
                    for jj in range(4):
                        nc.vector.transpose(
                            Uv[:, hr, hcc, jj], rh2v[:, hr, hcc, jj]
                        )

            # S = U + U^T
            S = espool.tile([128, 512], bf16, tag="S")
            Ureal = U[:].rearrange(
                "p (Hh Cc h j q) -> p Hh Cc h j q", Hh=2, Cc=2, h=32, j=4, q=2
            )
            for hr in range(2):
                psT = ppe.tile([128, 256], bf16, tag="ps_ut")
                for hcc in range(2):
                    nc.tensor.transpose(
                        psT[:, 128 * hcc : 128 * hcc + 128],
                        U[:, 512 * hcc + 256 * hr : 512 * hcc + 256 * hr + 128],
                        ident_bf[:],
                    )
                nc.vector.tensor_tensor(
                    out=S[:, 256 * hr : 256 * hr + 256].rearrange(
                        "p (Cc h j q) -> p Cc h j q", Cc=2, h=16, j=4
                    ),
                    in0=Ureal[:, hr, :, 0:16],
                    in1=psT[:].rearrange("p (Cc h j q) -> p Cc h j q", Cc=2, h=16, j=4),
                    op=OP.add,
                )

            rsum = espool.tile([128, 2], f32, tag="rsum")
            nc.vector.tensor_reduce(
                out=rsum[:], in_=S[:].rearrange("p (Hh x) -> p Hh x", Hh=2),
                axis=mybir.AxisListType.X, op=OP.add,
            )
            Ro = espool.tile([128, 512], f32, tag="Ro")
            for hr in range(2):
                db = 256 * hr + 128 * hr
                ob = 256 * hr + 128 * (1 - hr)
                tmp = espool.tile([128, 128], f32, tag="tmp_eye")
                nc.vector.tensor_tensor(
                    out=tmp[:], in0=S[:, db : db + 128], in1=ident_bf[:], op=OP.mult
                )
                sd = espool.tile([128, 1], f32, tag="sd")
                nc.vector.tensor_reduce(
                    out=sd[:], in_=tmp[:], axis=mybir.AxisListType.X, op=OP.add
                )
                tot = espool.tile([128, 1], f32, tag="tot")
                nc.vector.tensor_tensor(
                    out=tot[:], in0=rsum[:, hr : hr + 1], in1=sd[:], op=OP.add
                )
                dblk2 = espool.tile([128, 128], f32, tag="dblk2")
                nc.vector.tensor_scalar(
                    out=dblk2[:], in0=ident[:], scalar1=tot[:, 0:1], scalar2=None,
                    op0=OP.mult,
                )
                nc.vector.tensor_tensor(
                    out=Ro[:, db : db + 128], in0=dblk2[:], in1=S[:, db : db + 128],
                    op=OP.subtract,
                )
                nc.vector.tensor_scalar(
                    out=Ro[:, ob : ob + 128], in0=S[:, ob : ob + 128],
                    scalar1=-1.0, scalar2=None, op0=OP.mult,
                )
                nc.sync.dma_start(
                    out_ext[e, 128 * hr : 128 * hr + 128, :],
                    Ro[:, 256 * hr : 256 * hr + 256],
                )
    _split_excess_waits(nc)
    return nc


def kernel(**inputs):
    x = np.ascontiguousarray(inputs["x"], dtype=np.float32)
    L = np.ascontiguousarray(inputs["L"], dtype=np.float32)
    Aq = np.ascontiguousarray(inputs["Aq"], dtype=np.float32)
    Ak = np.ascontiguousarray(inputs["Ak"], dtype=np.float32)
    Av = np.ascontiguousarray(inputs["Av"], dtype=np.float32)
    Ao = np.ascontiguousarray(inputs["Ao"], dtype=np.float32)
    B = x.shape[0]
    M = 8
    BL = B // M

    from concourse.bass_utils import run_bass_kernel_spmd

    nc = build_graph()
    in_maps = []
    for c in range(M):
        in_maps.append({
            "x": x[c * BL : (c + 1) * BL],
            "L": L[c * BL : (c + 1) * BL],
            "Aq": Aq, "Ak": Ak, "Av": Av, "Ao": Ao,
        })
    res = run_bass_kernel_spmd(nc, in_maps, core_ids=list(range(M)))
    outs = [np.asarray(r["out"]) for r in res.results]
    return np.concatenate(outs, axis=0)


if __name__ == "__main__":
    nc = build_graph()
    print("graph built ok,", len(nc.instructions) if hasattr(nc, "instructions") else "")
